# revision 19
# baseline (speedup 1.0000x reference)
"""Trainium2 kernel for the nn_Circuit coupled-mode ODE problem.

Math: dA/dt = i*diag(omega + gamma*|A|^2) A + T2 A, integrated t in [0,2],
sampled at 200 points; A is (1024 batch, 64 modes) complex, padded with ones
for modes 48..63.  L = T2 + i*diag(omega) is constant, nearly skew-Hermitian,
with one stiff oscillatory eigenvalue (~288i).

Device algorithm: Strang splitting with the linear part EXACT via
host-precomputed matrix exponentials and the nonlinear part exact as a
per-element phase rotation A <- A*exp(i*gamma*h*|A|^2), one step per output
interval (h = 2/199).  With the half-shifted chain state z_k = E(h/2) y_k and
the rotation written as u = z*cc + P(z*ss) (P = re/im pair swap):

    z_{k+1} = E(h) u_k   = [E(h)]   p_k + [E(h)P]   qt_k
    y_{k+1} = E(h/2) u_k = [E(h/2)] p_k + [E(h/2)P] qt_k

where p = z*cc, qt = z*ss.  Both linear maps are evaluated as PSUM-accumulated
matmul pairs, so the pair swap and the final add never cost vector-engine ops.

State layout: (128 partitions, 128 batch) f32, partition p = 2j+c interleaving
re/im of mode j (|A|^2 needs only a pair-swap stream_shuffle).  The output is
written mode-major per core and transposed on the host during unsharding.

Sharding: pure data parallel, batch 1024 = 8 cores x 128.
"""

import os
import numpy as np

MODES = 64
INPUT_MODES = 48
BATCH = 1024
EVAL_PTS = 200
EPS = 1e-8
N_CORES = 8
B_LOC = BATCH // N_CORES  # 128
NT = EVAL_PTS - 1  # 199 intervals
DT = 2.0 / NT

_CACHE = {}


# ---------------------------------------------------------------------------
# host-side math
# ---------------------------------------------------------------------------

def _t2_like_reference(params, omega, kappa):
    """Reproduce the reference's float32 jax computation of T2 exactly."""
    import jax

    try:
        cpu = jax.devices("cpu")[0]
    except Exception:
        cpu = None

    import contextlib

    ctx = jax.default_device(cpu) if cpu is not None else contextlib.nullcontext()
    with ctx:
        import jax.numpy as jnp

        n = MODES
        p = jnp.asarray(params, dtype=jnp.float32)
        n_off = n * (n - 1) // 2
        iu = jnp.triu_indices(n, 1)
        off = p[:n_off] + 1j * p[n_off:2 * n_off]
        H = jnp.zeros((n, n), dtype=jnp.complex64).at[iu].set(off.astype(jnp.complex64))
        H = H + H.conj().T
        d = p[2 * n_off:]
        diag = jnp.concatenate([d, -jnp.sum(d, keepdims=True)])
        H = H + jnp.diag(diag.astype(jnp.complex64))
        U = jax.scipy.linalg.expm(1j * H)
        I = jnp.eye(n, dtype=jnp.complex64)
        M = U.T @ U
        mix = M @ jnp.linalg.inv(I - M + EPS * I)
        T2 = -jnp.asarray(kappa, dtype=jnp.float32) * (
            0.5 * jnp.eye(n, dtype=jnp.float32) + mix
        )
        T2_re = np.asarray(jnp.real(T2), dtype=np.float32)
        T2_im = np.asarray(jnp.imag(T2), dtype=np.float32)
    return T2_re, T2_im


def _expm(M):
    """Matrix exponential of a (diagonalizable) complex matrix via eig."""
    w, V = np.linalg.eig(M)
    return (V * np.exp(w)) @ np.linalg.inv(V)


def _big_il(C):
    """Complex (64,64) -> real (128,128) operator in the interleaved re/im basis."""
    A = np.zeros((2 * MODES, 2 * MODES), dtype=np.float64)
    Cr, Ci = C.real, C.imag
    A[0::2, 0::2] = Cr
    A[0::2, 1::2] = -Ci
    A[1::2, 0::2] = Ci
    A[1::2, 1::2] = Cr
    return A


def _host_precompute(A0, params, omega, kappa, nonlinearity):
    T2_re, T2_im = _t2_like_reference(params, omega, kappa)
    L = T2_re.astype(np.float64) + 1j * T2_im.astype(np.float64)
    L = L + 1j * np.diag(omega.astype(np.float64))

    A1 = _big_il(_expm(L * DT))         # full-step propagator E(h)
    A2 = _big_il(_expm(L * (DT / 2)))   # half-step propagator E(h/2)
    perm = np.arange(128) ^ 1           # re/im pair swap

    # lhsT arrangements: matmul computes lhsT.T @ rhs
    wEp = np.ascontiguousarray(A1.T, dtype=np.float32)
    wEq = np.ascontiguousarray(wEp[perm, :], dtype=np.float32)   # (A1 P)^T
    wYp = np.ascontiguousarray(A2.T, dtype=np.float32)
    wYq = np.ascontiguousarray(wYp[perm, :], dtype=np.float32)   # (A2 P)^T

    # initial state, interleaved mode-major: (128, BATCH)
    y0 = np.zeros((2 * MODES, BATCH), dtype=np.float64)
    y0[0:2 * INPUT_MODES:2, :] = A0[:, :, 0].astype(np.float64).T
    y0[1:2 * INPUT_MODES:2, :] = A0[:, :, 1].astype(np.float64).T
    y0[2 * INPUT_MODES::2, :] = 1.0
    z0 = (A2 @ y0).astype(np.float32)
    y0M = y0.astype(np.float32)

    gh = (nonlinearity.astype(np.float64) * DT)  # per-mode gamma*h
    # ss = sin(theta) signed (+ even partitions, - odd): u = z*cc + P(z*ss)
    sgn = np.tile([1.0, -1.0], MODES)
    sinscale = (np.repeat(gh, 2) * sgn).astype(np.float32).reshape(128, 1)
    # cc = 1 - theta^2/2 = msq * (-(gamma*h)^2/2) + 1
    ccscale = (-np.repeat(gh, 2) ** 2 / 2).astype(np.float32).reshape(128, 1)

    return dict(wEp=wEp, wEq=wEq, wYp=wYp, wYq=wYq, z0=z0, y0M=y0M,
                ccscale=ccscale, sinscale=sinscale)


# ---------------------------------------------------------------------------
# device kernel
# ---------------------------------------------------------------------------

def _build_nc():
    import concourse.bass as bass
    import concourse.bacc as bacc
    import concourse.tile as tile
    import concourse.mybir as mybir

    f32 = mybir.dt.float32
    bf16 = mybir.dt.bfloat16
    Sin = mybir.ActivationFunctionType.Sin
    Square = mybir.ActivationFunctionType.Square
    Copy = mybir.ActivationFunctionType.Copy
    add = mybir.AluOpType.add
    mult = mybir.AluOpType.mult
    P = 128
    pairswap = [i ^ 1 for i in range(32)]

    nc = bacc.Bacc("TRN2", target_bir_lowering=False, debug=False,
                   num_devices=N_CORES)

    wEp_d = nc.dram_tensor("wEp", [P, P], f32, kind="ExternalInput").ap()
    wEq_d = nc.dram_tensor("wEq", [P, P], f32, kind="ExternalInput").ap()
    wYp_d = nc.dram_tensor("wYp", [P, P], f32, kind="ExternalInput").ap()
    wYq_d = nc.dram_tensor("wYq", [P, P], f32, kind="ExternalInput").ap()
    z0_d = nc.dram_tensor("z0", [P, B_LOC], f32, kind="ExternalInput").ap()
    y0M_d = nc.dram_tensor("y0M", [P, B_LOC], f32, kind="ExternalInput").ap()
    ccscale_d = nc.dram_tensor("ccscale", [P, 1], f32, kind="ExternalInput").ap()
    sinscale_d = nc.dram_tensor("sinscale", [P, 1], f32, kind="ExternalInput").ap()
    # mode-major output: (t, 2j+c, b_local); host transposes while unsharding
    out_d = nc.dram_tensor("out", [EVAL_PTS, P, B_LOC], f32, kind="ExternalOutput").ap()

    with tile.TileContext(nc) as tc:
        with (
            tc.tile_pool(name="const", bufs=1) as cpool,
            tc.tile_pool(name="nl", bufs=4) as npool,
            tc.tile_pool(name="oy", bufs=4) as opool,
            tc.tile_pool(name="pz", bufs=2, space="PSUM") as pzpool,
            tc.tile_pool(name="py", bufs=2, space="PSUM") as pypool,
            tc.tile_pool(name="pd", bufs=1, space="PSUM") as pdpool,
        ):
            wEp_t = cpool.tile([P, P], f32, tag="wEp")
            wEq_t = cpool.tile([P, P], f32, tag="wEq")
            wYp_t = cpool.tile([P, P], f32, tag="wYp")
            wYq_t = cpool.tile([P, P], f32, tag="wYq")
            ccsc_t = cpool.tile([P, 1], f32, tag="ccsc")
            sinsc_t = cpool.tile([P, 1], f32, tag="sinsc")
            nc.sync.dma_start(wEp_t[:], wEp_d[:])
            nc.sync.dma_start(wEq_t[:], wEq_d[:])
            nc.sync.dma_start(wYp_t[:], wYp_d[:])
            nc.sync.dma_start(wYq_t[:], wYq_d[:])
            nc.sync.dma_start(ccsc_t[:], ccscale_d[:])
            nc.sync.dma_start(sinsc_t[:], sinscale_d[:])

            # t=0 output: pass-through of the initial state (mode-major)
            y0_t = opool.tile([P, B_LOC], f32, tag="yc")
            nc.sync.dma_start(y0_t[:], y0M_d[:])
            nc.sync.dma_start(out_d[0], y0_t[:])

            dscr = pdpool.tile([P, B_LOC], f32, tag="dscr")
            dW_t = cpool.tile([P, P], bf16, tag="dW")
            nc.vector.memset(dW_t[:], 1.0)

            def nl_rotation(zsrc, from_sbuf):
                """Return (p, qt) SBUF tiles: p = z*cc, qt = z*ss.

                All on V (cross-engine PE->ACT->V hops cost ~300-500ns each in
                semaphore latency; V-FIFO keeps ops back-to-back).  The Sin LUT
                (ACT) overlaps V's msq/cc ops.  Two dummy matmuls chained on
                mid-rotation tiles keep the PE HAM-warm through the ~2us
                vector phase so the chain matmuls run at 2.4 GHz.
                """
                s2 = npool.tile([P, B_LOC], bf16, tag="s2")
                s2sw = npool.tile([P, B_LOC], bf16, tag="s2sw")
                m2 = npool.tile([P, B_LOC], bf16, tag="m2")
                msq = npool.tile([P, B_LOC], bf16, tag="msq")
                cc = npool.tile([P, B_LOC], f32, tag="cc")
                ssp = npool.tile([P, B_LOC], f32, tag="ssp")
                pp = npool.tile([P, B_LOC], f32, tag="pp")
                qt = npool.tile([P, B_LOC], f32, tag="qt")
                if from_sbuf:
                    zc = zsrc
                else:
                    zct = npool.tile([P, B_LOC], f32, tag="zc")
                    nc.vector.tensor_copy(zct[:], zsrc)
                    zc = zct[:]
                nc.vector.tensor_tensor(s2[:], zc, zc, mult)
                nc.vector.stream_shuffle(s2sw[:], s2[:], pairswap)
                # dummy warm-up matmuls: bf16 single-pass, chained on
                # mid-rotation tiles so they fire inside the PE-idle window
                nc.tensor.matmul(dscr[:], dW_t[:], s2[:], start=True, stop=True)
                nc.vector.tensor_tensor(m2[:], s2[:], s2sw[:], add)
                nc.tensor.matmul(dscr[:], dW_t[:], m2[:], start=True, stop=True)
                nc.scalar.activation(ssp[:], m2[:], Sin, scale=sinsc_t[:])
                nc.vector.tensor_tensor(msq[:], m2[:], m2[:], mult)
                nc.vector.tensor_scalar(cc[:], msq[:], ccsc_t[:], 1.0, mult, add)
                nc.vector.tensor_tensor(pp[:], zc, cc[:], mult)
                nc.vector.tensor_tensor(qt[:], zc, ssp[:], mult)
                return pp, qt

            # ---- initial rotation from z0 (SBUF) ----
            z0_t = npool.tile([P, B_LOC], f32, tag="z0src")
            nc.sync.dma_start(z0_t[:], z0_d[:])
            pp, qt = nl_rotation(z0_t[:], from_sbuf=True)

            # ---- main loop ----
            # PSUM discipline: z read by ACT (Square) then V (pp/qt), ordered
            # by the dependency chain; yps read only by ACT (yc copy).
            for k in range(NT):
                if k < NT - 1:
                    z = pzpool.tile([P, B_LOC], f32, tag="z")
                    nc.tensor.matmul(z[:], wEp_t[:], pp[:], start=True, stop=False)
                    nc.tensor.matmul(z[:], wEq_t[:], qt[:], start=False, stop=True)

                yps = pypool.tile([P, B_LOC], f32, tag="yps")
                nc.tensor.matmul(yps[:], wYp_t[:], pp[:], start=True, stop=False)
                nc.tensor.matmul(yps[:], wYq_t[:], qt[:], start=False, stop=True)
                yc = opool.tile([P, B_LOC], f32, tag="yc")
                nc.scalar.activation(yc[:], yps[:], Copy)
                nc.sync.dma_start(out_d[k + 1], yc[:])

                if k == NT - 1:
                    break
                pp, qt = nl_rotation(z[:], from_sbuf=False)

    nc.compile()
    return nc


def _get_compiled():
    if "nc" not in _CACHE:
        _CACHE["nc"] = _build_nc()
    return _CACHE["nc"]


def _run(host, trace=False, tmpdir=None):
    from concourse.bass_utils import run_bass_kernel_spmd

    nc = _get_compiled()
    in_maps = []
    for i in range(N_CORES):
        sl = slice(i * B_LOC, (i + 1) * B_LOC)
        in_maps.append({
            "wEp": host["wEp"],
            "wEq": host["wEq"],
            "wYp": host["wYp"],
            "wYq": host["wYq"],
            "z0": np.ascontiguousarray(host["z0"][:, sl]),
            "y0M": np.ascontiguousarray(host["y0M"][:, sl]),
            "ccscale": host["ccscale"],
            "sinscale": host["sinscale"],
        })
    res = run_bass_kernel_spmd(nc, in_maps, list(range(N_CORES)), trace=trace,
                               tmpdir=tmpdir)
    full = np.empty((EVAL_PTS, BATCH, MODES, 2), dtype=np.float32)
    for i in range(N_CORES):
        sl = slice(i * B_LOC, (i + 1) * B_LOC)
        # core output is (t, 2j+c, b_local) -> (t, b_local, j, c)
        arr = res.results[i]["out"]
        full[:, sl, :, :] = arr.transpose(0, 2, 1).reshape(EVAL_PTS, B_LOC, MODES, 2)
    return full, res


def kernel(A0, params, omega, kappa, nonlinearity):
    A0 = np.asarray(A0, dtype=np.float32)
    params = np.asarray(params, dtype=np.float32)
    omega = np.asarray(omega, dtype=np.float32)
    kappa = np.asarray(kappa, dtype=np.float32)
    nonlinearity = np.asarray(nonlinearity, dtype=np.float32)

    host = _host_precompute(A0, params, omega, kappa, nonlinearity)
    full, _ = _run(host, trace=False)
    return full


# revision 20
# speedup vs baseline: 1.1553x; 1.1553x over previous
"""Trainium2 kernel for the nn_Circuit coupled-mode ODE problem.

Math: dA/dt = i*diag(omega + gamma*|A|^2) A + T2 A, integrated t in [0,2],
sampled at 200 points; A is (1024 batch, 64 modes) complex, padded with ones
for modes 48..63.  L = T2 + i*diag(omega) is constant, nearly skew-Hermitian,
with one stiff oscillatory eigenvalue (~288i).

Device algorithm: Strang splitting with the linear part EXACT via
host-precomputed matrix exponentials and the nonlinear part exact as a
per-element phase rotation A <- A*exp(i*gamma*h*|A|^2), one step per output
interval (h = 2/199).  With the half-shifted chain state z_k = E(h/2) y_k and
the rotation written as u = z*cc + P(z*ss) (P = re/im pair swap):

    z_{k+1} = E(h) u_k   = [E(h)]   p_k + [E(h)P]   qt_k
    y_{k+1} = E(h/2) u_k = [E(h/2)] p_k + [E(h/2)P] qt_k

where p = z*cc, qt = z*ss.  Both linear maps are evaluated as PSUM-accumulated
matmul pairs, so the pair swap and the final add never cost vector-engine ops.

State layout: (128 partitions, 128 batch) f32, partition p = 2j+c interleaving
re/im of mode j (|A|^2 needs only a pair-swap stream_shuffle).  The output is
written mode-major per core and transposed on the host during unsharding.

Sharding: pure data parallel, batch 1024 = 8 cores x 128.
"""

import os
import numpy as np

MODES = 64
INPUT_MODES = 48
BATCH = 1024
EVAL_PTS = 200
EPS = 1e-8
N_CORES = 8
B_LOC = BATCH // N_CORES  # 128
NT = EVAL_PTS - 1  # 199 intervals
DT = 2.0 / NT

_CACHE = {}


# ---------------------------------------------------------------------------
# host-side math
# ---------------------------------------------------------------------------

def _t2_like_reference(params, omega, kappa):
    """Reproduce the reference's float32 jax computation of T2 exactly."""
    import jax

    try:
        cpu = jax.devices("cpu")[0]
    except Exception:
        cpu = None

    import contextlib

    ctx = jax.default_device(cpu) if cpu is not None else contextlib.nullcontext()
    with ctx:
        import jax.numpy as jnp

        n = MODES
        p = jnp.asarray(params, dtype=jnp.float32)
        n_off = n * (n - 1) // 2
        iu = jnp.triu_indices(n, 1)
        off = p[:n_off] + 1j * p[n_off:2 * n_off]
        H = jnp.zeros((n, n), dtype=jnp.complex64).at[iu].set(off.astype(jnp.complex64))
        H = H + H.conj().T
        d = p[2 * n_off:]
        diag = jnp.concatenate([d, -jnp.sum(d, keepdims=True)])
        H = H + jnp.diag(diag.astype(jnp.complex64))
        U = jax.scipy.linalg.expm(1j * H)
        I = jnp.eye(n, dtype=jnp.complex64)
        M = U.T @ U
        mix = M @ jnp.linalg.inv(I - M + EPS * I)
        T2 = -jnp.asarray(kappa, dtype=jnp.float32) * (
            0.5 * jnp.eye(n, dtype=jnp.float32) + mix
        )
        T2_re = np.asarray(jnp.real(T2), dtype=np.float32)
        T2_im = np.asarray(jnp.imag(T2), dtype=np.float32)
    return T2_re, T2_im


def _expm(M):
    """Matrix exponential of a (diagonalizable) complex matrix via eig."""
    w, V = np.linalg.eig(M)
    return (V * np.exp(w)) @ np.linalg.inv(V)


def _big_il(C):
    """Complex (64,64) -> real (128,128) operator in the interleaved re/im basis."""
    A = np.zeros((2 * MODES, 2 * MODES), dtype=np.float64)
    Cr, Ci = C.real, C.imag
    A[0::2, 0::2] = Cr
    A[0::2, 1::2] = -Ci
    A[1::2, 0::2] = Ci
    A[1::2, 1::2] = Cr
    return A


def _host_precompute(A0, params, omega, kappa, nonlinearity):
    T2_re, T2_im = _t2_like_reference(params, omega, kappa)
    L = T2_re.astype(np.float64) + 1j * T2_im.astype(np.float64)
    L = L + 1j * np.diag(omega.astype(np.float64))

    A1 = _big_il(_expm(L * DT))         # full-step propagator E(h)
    A2 = _big_il(_expm(L * (DT / 2)))   # half-step propagator E(h/2)
    perm = np.arange(128) ^ 1           # re/im pair swap

    # lhsT arrangements: matmul computes lhsT.T @ rhs
    wEp = np.ascontiguousarray(A1.T, dtype=np.float32)
    wEq = np.ascontiguousarray(wEp[perm, :], dtype=np.float32)   # (A1 P)^T
    wYp = np.ascontiguousarray(A2.T, dtype=np.float32)
    wYq = np.ascontiguousarray(wYp[perm, :], dtype=np.float32)   # (A2 P)^T

    # initial state, interleaved mode-major: (128, BATCH)
    y0 = np.zeros((2 * MODES, BATCH), dtype=np.float64)
    y0[0:2 * INPUT_MODES:2, :] = A0[:, :, 0].astype(np.float64).T
    y0[1:2 * INPUT_MODES:2, :] = A0[:, :, 1].astype(np.float64).T
    y0[2 * INPUT_MODES::2, :] = 1.0
    z0 = (A2 @ y0).astype(np.float32)
    y0M = y0.astype(np.float32)

    gh = (nonlinearity.astype(np.float64) * DT)  # per-mode gamma*h
    # ss = sin(theta) signed (+ even partitions, - odd): u = z*cc + P(z*ss)
    sgn = np.tile([1.0, -1.0], MODES)
    sinscale = (np.repeat(gh, 2) * sgn).astype(np.float32).reshape(128, 1)
    # cc = 1 - theta^2/2 = msq * (-(gamma*h)^2/2) + 1
    ccscale = (-np.repeat(gh, 2) ** 2 / 2).astype(np.float32).reshape(128, 1)

    return dict(wEp=wEp, wEq=wEq, wYp=wYp, wYq=wYq, z0=z0, y0M=y0M,
                ccscale=ccscale, sinscale=sinscale)


# ---------------------------------------------------------------------------
# device kernel
# ---------------------------------------------------------------------------

def _build_nc():
    import concourse.bass as bass
    import concourse.bacc as bacc
    import concourse.tile as tile
    import concourse.mybir as mybir

    f32 = mybir.dt.float32
    bf16 = mybir.dt.bfloat16
    Sin = mybir.ActivationFunctionType.Sin
    Square = mybir.ActivationFunctionType.Square
    Copy = mybir.ActivationFunctionType.Copy
    add = mybir.AluOpType.add
    mult = mybir.AluOpType.mult
    P = 128
    pairswap = [i ^ 1 for i in range(32)]

    nc = bacc.Bacc("TRN2", target_bir_lowering=False, debug=False,
                   num_devices=N_CORES)

    wEp_d = nc.dram_tensor("wEp", [P, P], f32, kind="ExternalInput").ap()
    wEq_d = nc.dram_tensor("wEq", [P, P], f32, kind="ExternalInput").ap()
    wYp_d = nc.dram_tensor("wYp", [P, P], f32, kind="ExternalInput").ap()
    wYq_d = nc.dram_tensor("wYq", [P, P], f32, kind="ExternalInput").ap()
    z0_d = nc.dram_tensor("z0", [P, B_LOC], f32, kind="ExternalInput").ap()
    y0M_d = nc.dram_tensor("y0M", [P, B_LOC], f32, kind="ExternalInput").ap()
    ccscale_d = nc.dram_tensor("ccscale", [P, 1], f32, kind="ExternalInput").ap()
    sinscale_d = nc.dram_tensor("sinscale", [P, 1], f32, kind="ExternalInput").ap()
    # mode-major output: (t, 2j+c, b_local); host transposes while unsharding
    out_d = nc.dram_tensor("out", [EVAL_PTS, P, B_LOC], f32, kind="ExternalOutput").ap()

    with tile.TileContext(nc) as tc:
        with (
            tc.tile_pool(name="const", bufs=1) as cpool,
            tc.tile_pool(name="nl", bufs=4) as npool,
            tc.tile_pool(name="oy", bufs=4) as opool,
            tc.tile_pool(name="pz", bufs=2, space="PSUM") as pzpool,
            tc.tile_pool(name="py", bufs=2, space="PSUM") as pypool,
            tc.tile_pool(name="pd", bufs=1, space="PSUM") as pdpool,
        ):
            wEp_t = cpool.tile([P, P], f32, tag="wEp")
            wEq_t = cpool.tile([P, P], f32, tag="wEq")
            wYp_t = cpool.tile([P, P], f32, tag="wYp")
            wYq_t = cpool.tile([P, P], f32, tag="wYq")
            ccsc_t = cpool.tile([P, 1], f32, tag="ccsc")
            sinsc_t = cpool.tile([P, 1], f32, tag="sinsc")
            nc.sync.dma_start(wEp_t[:], wEp_d[:])
            nc.sync.dma_start(wEq_t[:], wEq_d[:])
            nc.sync.dma_start(wYp_t[:], wYp_d[:])
            nc.sync.dma_start(wYq_t[:], wYq_d[:])
            nc.sync.dma_start(ccsc_t[:], ccscale_d[:])
            nc.sync.dma_start(sinsc_t[:], sinscale_d[:])

            # t=0 output: pass-through of the initial state (mode-major)
            y0_t = opool.tile([P, B_LOC], f32, tag="yc")
            nc.sync.dma_start(y0_t[:], y0M_d[:])
            nc.sync.dma_start(out_d[0], y0_t[:])

            dscr = pdpool.tile([P, B_LOC], f32, tag="dscr")
            dW_t = cpool.tile([P, P], bf16, tag="dW")
            nc.vector.memset(dW_t[:], 1.0)

            def nl_rotation(zsrc, from_sbuf):
                """Return (p, qt) SBUF tiles: p = z*cc, qt = z*ss.

                All on V (cross-engine PE->ACT->V hops cost ~300-500ns each in
                semaphore latency; V-FIFO keeps ops back-to-back).  The Sin LUT
                (ACT) overlaps V's msq/cc ops.  Two dummy matmuls chained on
                mid-rotation tiles keep the PE HAM-warm through the ~2us
                vector phase so the chain matmuls run at 2.4 GHz.
                """
                s2 = npool.tile([P, B_LOC], bf16, tag="s2")
                s2sw = npool.tile([P, B_LOC], bf16, tag="s2sw")
                m2 = npool.tile([P, B_LOC], bf16, tag="m2")
                msq = npool.tile([P, B_LOC], bf16, tag="msq")
                cc = npool.tile([P, B_LOC], f32, tag="cc")
                ssp = npool.tile([P, B_LOC], f32, tag="ssp")
                pp = npool.tile([P, B_LOC], f32, tag="pp")
                qt = npool.tile([P, B_LOC], f32, tag="qt")
                if from_sbuf:
                    zc = zsrc
                else:
                    zct = npool.tile([P, B_LOC], f32, tag="zc")
                    nc.vector.tensor_copy(zct[:], zsrc)
                    zc = zct[:]
                nc.vector.tensor_tensor(s2[:], zc, zc, mult)
                # dummy warm-up matmuls (f32, 2 passes each): chained on early
                # and mid-rotation f32 tiles so they fire inside the PE-idle
                # window and keep the HAM clock gate at 2.4 GHz
                nc.tensor.matmul(dscr[:], wEp_t[:], zc, start=True, stop=True)
                nc.vector.stream_shuffle(s2sw[:], s2[:], pairswap)
                nc.vector.tensor_tensor(m2[:], s2[:], s2sw[:], add)
                nc.scalar.activation(ssp[:], m2[:], Sin, scale=sinsc_t[:])
                nc.vector.tensor_tensor(msq[:], m2[:], m2[:], mult)
                nc.vector.tensor_scalar(cc[:], msq[:], ccsc_t[:], 1.0, mult, add)
                nc.tensor.matmul(dscr[:], wEp_t[:], ssp[:], start=True, stop=True)
                nc.vector.tensor_tensor(pp[:], zc, cc[:], mult)
                nc.vector.tensor_tensor(qt[:], zc, ssp[:], mult)
                return pp, qt

            # ---- initial rotation from z0 (SBUF) ----
            z0_t = npool.tile([P, B_LOC], f32, tag="z0src")
            nc.sync.dma_start(z0_t[:], z0_d[:])
            pp, qt = nl_rotation(z0_t[:], from_sbuf=True)

            # ---- main loop ----
            # PSUM discipline: z read by ACT (Square) then V (pp/qt), ordered
            # by the dependency chain; yps read only by ACT (yc copy).
            for k in range(NT):
                if k < NT - 1:
                    z = pzpool.tile([P, B_LOC], f32, tag="z")
                    nc.tensor.matmul(z[:], wEp_t[:], pp[:], start=True, stop=False)
                    nc.tensor.matmul(z[:], wEq_t[:], qt[:], start=False, stop=True)

                yps = pypool.tile([P, B_LOC], f32, tag="yps")
                nc.tensor.matmul(yps[:], wYp_t[:], pp[:], start=True, stop=False)
                nc.tensor.matmul(yps[:], wYq_t[:], qt[:], start=False, stop=True)
                yc = opool.tile([P, B_LOC], f32, tag="yc")
                nc.scalar.activation(yc[:], yps[:], Copy)
                nc.sync.dma_start(out_d[k + 1], yc[:])

                if k == NT - 1:
                    break
                pp, qt = nl_rotation(z[:], from_sbuf=False)

    nc.compile()
    return nc


def _get_compiled():
    if "nc" not in _CACHE:
        _CACHE["nc"] = _build_nc()
    return _CACHE["nc"]


def _run(host, trace=False, tmpdir=None):
    from concourse.bass_utils import run_bass_kernel_spmd

    nc = _get_compiled()
    in_maps = []
    for i in range(N_CORES):
        sl = slice(i * B_LOC, (i + 1) * B_LOC)
        in_maps.append({
            "wEp": host["wEp"],
            "wEq": host["wEq"],
            "wYp": host["wYp"],
            "wYq": host["wYq"],
            "z0": np.ascontiguousarray(host["z0"][:, sl]),
            "y0M": np.ascontiguousarray(host["y0M"][:, sl]),
            "ccscale": host["ccscale"],
            "sinscale": host["sinscale"],
        })
    res = run_bass_kernel_spmd(nc, in_maps, list(range(N_CORES)), trace=trace,
                               tmpdir=tmpdir)
    full = np.empty((EVAL_PTS, BATCH, MODES, 2), dtype=np.float32)
    for i in range(N_CORES):
        sl = slice(i * B_LOC, (i + 1) * B_LOC)
        # core output is (t, 2j+c, b_local) -> (t, b_local, j, c)
        arr = res.results[i]["out"]
        full[:, sl, :, :] = arr.transpose(0, 2, 1).reshape(EVAL_PTS, B_LOC, MODES, 2)
    return full, res


def kernel(A0, params, omega, kappa, nonlinearity):
    A0 = np.asarray(A0, dtype=np.float32)
    params = np.asarray(params, dtype=np.float32)
    omega = np.asarray(omega, dtype=np.float32)
    kappa = np.asarray(kappa, dtype=np.float32)
    nonlinearity = np.asarray(nonlinearity, dtype=np.float32)

    host = _host_precompute(A0, params, omega, kappa, nonlinearity)
    full, _ = _run(host, trace=False)
    return full
